# revision 11
# baseline (speedup 1.0000x reference)
"""DeepSeek MoE layer on 8 Trainium2 NeuronCores (Bass/Tile).

Strategy: exploit top-8-of-32 routing sparsity. The host computes the
routing (float64 numpy replica of the reference's grouped top-k), gathers
each expert's ~128 routed tokens into a compact batch, and the device only
runs the expert MLPs on those tokens (~1/4 of the dense FLOPs). Combine
weights commute with the down-projection (per-token scaling), so they are
applied on the host during scatter-add — the device is a pure GEMM pipeline.

Sharding: expert parallelism. Core c owns routed experts 4c..4c+3 (sorted
into capacity slots by token count) and a 256-wide slice of the shared
experts' intermediate dim. Weights are re-tiled on the host into
partition-major blocks so every weight DMA is a single 1-2 MB transfer with
16 KB contiguous runs per partition. Expert matmuls run in bf16 with fp32
PSUM accumulation; outputs return in bf16 and are combined in fp32 on host.
"""

import sys

sys.path.insert(0, "/opt/trn_rl_repo")

import numpy as np
import ml_dtypes

import concourse.bass as bass  # noqa: F401
import concourse.mybir as mybir
import concourse.tile as tile
from concourse import bacc
from concourse.bass_utils import run_bass_kernel_spmd

F32 = mybir.dt.float32
BF16 = mybir.dt.bfloat16
AF = mybir.ActivationFunctionType

# Problem constants (hardcoded per contract).
T = 512       # tokens
H = 2048      # hidden
I = 1024      # moe intermediate
E = 32        # routed experts
K = 8         # experts per token
NG = 8        # routing groups
TG = 4        # top-k groups
SCALE = 2.5   # routed scaling factor
NCORES = 8
EL = E // NCORES          # local experts per core = 4
SI = 256                  # shared-intermediate slice per core (2*1024/8)
P = 128
HK = H // P               # 16 k-tiles over hidden
IK = I // P               # 8 k-tiles over intermediate
BF = ml_dtypes.bfloat16


# ---------------------------------------------------------------- routing --
def host_routing(x, gate_w, gate_bias):
    """Float64 numpy replica of reference._grouped_topk. Returns
    (weights [T,K] f64, ids [T,K] int64)."""
    xl = np.asarray(x, np.float64)
    logits = xl @ np.asarray(gate_w, np.float64).T          # [T,E]
    s = 1.0 / (1.0 + np.exp(-logits))
    sc = s + np.asarray(gate_bias, np.float64)[None, :]
    grp = sc.reshape(T, NG, E // NG)
    top2 = np.sort(grp, axis=2)[:, :, -2:].sum(axis=2)      # [T,NG]
    gidx = np.argsort(-top2, axis=1, kind="stable")[:, :TG]
    gmask = np.zeros((T, NG), bool)
    gmask[np.arange(T)[:, None], gidx] = True
    emask = np.repeat(gmask, E // NG, axis=1)               # [T,E]
    masked = np.where(emask, sc, -np.inf)
    ids = np.argsort(-masked, axis=1, kind="stable")[:, :K]  # [T,K]
    w = np.take_along_axis(s, ids, axis=1)
    w = w / w.sum(axis=1, keepdims=True) * SCALE
    return w, ids


def _pmajor(a2d, cols):
    """[R, C] -> [128, R//128, C'] partition-major blocks where the C dim is
    pre-split into col groups of `cols`: returns [C//cols, 128, R//128, cols]."""
    r, c = a2d.shape
    return np.ascontiguousarray(
        a2d.reshape(r // P, P, c // cols, cols).transpose(2, 1, 0, 3))


def prepare(hidden_states, gate_w, gate_bias, w_gate_up, w_down,
            shared_w_gate_up, shared_w_down):
    """Host-side routing + gather + weight re-tiling.
    Returns (caps, in_maps, meta)."""
    x = np.asarray(hidden_states, np.float32)
    w, ids = host_routing(x, gate_w, gate_bias)

    toks = [np.nonzero((ids == e).any(axis=1))[0] for e in range(E)]
    wts = []
    for e in range(E):
        sel = ids[toks[e]] == e                     # [cnt, K] one-hot-ish
        wts.append((w[toks[e]] * sel).sum(axis=1))  # [cnt]
    cnts = np.array([len(t) for t in toks])

    # slot assignment: global sort by count desc; slot j holds ranks
    # 8j..8j+7 so cap_j = count of rank 8j (minimal padding), and every
    # core gets one expert from each rank band (balanced work).
    order = np.argsort(-cnts, kind="stable")
    slot_exp = np.zeros((NCORES, EL), np.int64)
    for j in range(EL):
        slot_exp[:, j] = order[j * NCORES:(j + 1) * NCORES]
    caps = tuple(
        max(16, int(np.ceil(cnts[slot_exp[:, j]].max() / 8.0) * 8))
        for j in range(EL))

    wgu = np.asarray(w_gate_up, np.float32)
    wd = np.asarray(w_down, np.float32)
    swgu = np.asarray(shared_w_gate_up, np.float32)
    swd = np.asarray(shared_w_down, np.float32)

    xT = x.T                                        # [H, T]
    xTb_r = _pmajor(xT.astype(BF), 512)[0]          # [128, HK, 512]

    in_maps = []
    for c in range(NCORES):
        m = {"xTb": xTb_r}
        # shared gate_up slice: [gate 256 | up 256] cols -> [128, HK, 512]
        sw = np.concatenate([
            swgu[:, c * SI:(c + 1) * SI],
            swgu[:, 2 * I + c * SI: 2 * I + (c + 1) * SI]], axis=1)
        m["swgu"] = _pmajor(sw.astype(BF), 512)[0]
        # shared down slice rows -> [128, 4hq, 2i2, 512]
        sd = swd[c * SI:(c + 1) * SI, :].astype(BF)  # [256, 2048]
        m["swd"] = np.ascontiguousarray(
            sd.reshape(2, P, 4, 512).transpose(1, 2, 0, 3))
        wgu_r = np.empty((EL, 4, P, HK, 512), BF)
        wd_r = np.empty((EL, 4, P, IK, 512), BF)
        for j in range(EL):
            e = slot_exp[c, j]
            wgu_r[j] = _pmajor(wgu[e].astype(BF), 512)   # [4q, 128, HK, 512]
            wd_r[j] = _pmajor(wd[e].astype(BF), 512)     # [4hq, 128, IK, 512]
            xe = xT[:, toks[e]].astype(BF)               # [H, cnt]
            xg = np.zeros((P, HK, caps[j]), BF)
            xg[:, :, :cnts[e]] = xe.reshape(HK, P, -1).transpose(1, 0, 2)
            m[f"xg{j}"] = xg
        m["wgu"] = wgu_r
        m["wd"] = wd_r
        in_maps.append(m)

    meta = {"toks": toks, "wts": wts, "slot_exp": slot_exp, "cnts": cnts}
    return caps, in_maps, meta


def combine(results, caps, meta):
    """Scatter-add per-expert outputs (scaled by combine weights) + shared
    partials into the full [T, H] output."""
    acc = np.zeros((H, T), np.float32)
    for c in range(NCORES):
        r = results[c]
        acc += np.asarray(r["outS"], np.float32).transpose(1, 0, 2).reshape(H, T)
        for j in range(EL):
            e = meta["slot_exp"][c, j]
            tk = meta["toks"][e]
            if len(tk) == 0:
                continue
            y = np.asarray(r[f"y{j}"], np.float32).transpose(1, 0, 2)
            y = y.reshape(H, caps[j])[:, :len(tk)]
            acc[:, tk] += y * meta["wts"][e][None, :].astype(np.float32)
    return np.ascontiguousarray(acc.T)


# ----------------------------------------------------------------- device --
def _build_body(tc, d, pools, caps):
    nc = tc.nc
    sb, work, wstream, ps = pools

    # expert-0 gather arrives first so PE work starts after minimal DMA
    xg = [sb.tile([P, HK, caps[j]], BF16, name=f"xg{j}") for j in range(EL)]
    nc.sync.dma_start(xg[0][:], d["xg0"][:])

    def gate_up(j, split_first=False):
        cap = caps[j]
        actw = sb.tile([P, IK, cap], BF16, name=f"actw{j}")
        sg = work.tile([P, IK, cap], F32, tag="sg")
        for q in range(4):
            wq = wstream.tile([P, HK, 512], BF16, tag="w")
            if split_first and q == 0:
                # smaller pieces so the k-loop can start sooner
                for kg in range(4):
                    nc.sync.dma_start(wq[:, 4 * kg:4 * (kg + 1), :],
                                      d["wgu"][j, q, :, 4 * kg:4 * (kg + 1), :])
            else:
                nc.sync.dma_start(wq[:], d["wgu"][j, q, :, :, :])
            pps = [ps.tile([P, cap], F32, tag="mm", name=f"pps{i}")
                   for i in range(4)]
            for k in range(HK):
                for i in range(4):
                    nc.tensor.matmul(pps[i][:], wq[:, k, i * P:(i + 1) * P],
                                     xg[j][:, k, :],
                                     start=(k == 0), stop=(k == HK - 1))
            if q < 2:
                for i in range(4):
                    it = 4 * q + i
                    sgm = work.tile([P, cap], F32, tag="sgm")
                    nc.scalar.activation(sgm[:], pps[i][:], AF.Sigmoid)
                    nc.vector.tensor_mul(sg[:, it, :], sgm[:], pps[i][:])
            else:
                for i in range(4):
                    it = 4 * (q - 2) + i
                    nc.vector.tensor_mul(actw[:, it, :], sg[:, it, :], pps[i][:])
        return actw

    def down(j, actw):
        cap = caps[j]
        y = work.tile([P, HK, cap], BF16, tag="y")
        for hq in range(4):
            wq = wstream.tile([P, IK, 512], BF16, tag="w")
            nc.sync.dma_start(wq[:], d["wd"][j, hq, :, :, :])
            ppd = [ps.tile([P, cap], F32, tag="mm", name=f"ppd{h}")
                   for h in range(4)]
            for i2 in range(IK):
                for h in range(4):
                    nc.tensor.matmul(ppd[h][:], wq[:, i2, h * P:(h + 1) * P],
                                     actw[:, i2, :],
                                     start=(i2 == 0), stop=(i2 == IK - 1))
            for h in range(4):
                nc.vector.tensor_copy(y[:, 4 * hq + h, :], ppd[h][:])
        nc.sync.dma_start(d[f"y{j}"][:], y[:])

    actw0 = gate_up(0, split_first=True)

    # remaining resident loads after the critical path is primed
    for j in range(1, EL):
        nc.sync.dma_start(xg[j][:], d[f"xg{j}"][:])
    xTb = sb.tile([P, HK, 512], BF16, name="xTb")
    nc.sync.dma_start(xTb[:], d["xTb"][:])
    swdt = sb.tile([P, 4, 2, 512], BF16, name="swdt")
    nc.sync.dma_start(swdt[:], d["swd"][:])

    # shared experts gate_up (fills PE while expert weights stream)
    swt = wstream.tile([P, HK, 512], BF16, tag="w")
    nc.sync.dma_start(swt[:], d["swgu"][:])
    pss = [ps.tile([P, T], F32, tag="mm", name=f"pss{i}") for i in range(4)]
    for k in range(HK):
        for i in range(4):
            nc.tensor.matmul(pss[i][:], swt[:, k, i * P:(i + 1) * P],
                             xTb[:, k, :], start=(k == 0), stop=(k == HK - 1))
    acts = sb.tile([P, 2, T], BF16, name="acts")
    for t in range(2):
        sst = work.tile([P, T], F32, tag="sst")
        nc.scalar.activation(sst[:], pss[t][:], AF.Sigmoid)
        nc.vector.tensor_mul(sst[:], sst[:], pss[t][:])
        nc.vector.tensor_mul(acts[:, t, :], sst[:], pss[2 + t][:])

    down(0, actw0)
    for j in range(1, EL - 1):
        actw = gate_up(j)
        down(j, actw)
    actw3 = gate_up(EL - 1)

    # shared down before the last expert's down so the tail stays short
    outS = sb.tile([P, HK, 512], BF16, name="outS")
    for hq in range(4):
        ppd = [ps.tile([P, T], F32, tag="mm", name=f"pps{h}") for h in range(4)]
        for i2 in range(2):
            for h in range(4):
                nc.tensor.matmul(ppd[h][:], swdt[:, hq, i2, h * P:(h + 1) * P],
                                 acts[:, i2, :],
                                 start=(i2 == 0), stop=(i2 == 1))
        for h in range(4):
            nc.vector.tensor_copy(outS[:, 4 * hq + h, :], ppd[h][:])
        nc.sync.dma_start(d["outS"][:, 4 * hq:4 * (hq + 1), :],
                          outS[:, 4 * hq:4 * (hq + 1), :])

    down(EL - 1, actw3)


def build_nc(caps, repeat=1):
    nc = bacc.Bacc("TRN2", target_bir_lowering=False, debug=False,
                   num_devices=NCORES)
    d = {
        "xTb": nc.dram_tensor("xTb", [P, HK, 512], BF16, kind="ExternalInput").ap(),
        "swgu": nc.dram_tensor("swgu", [P, HK, 512], BF16, kind="ExternalInput").ap(),
        "swd": nc.dram_tensor("swd", [P, 4, 2, 512], BF16, kind="ExternalInput").ap(),
        "wgu": nc.dram_tensor("wgu", [EL, 4, P, HK, 512], BF16, kind="ExternalInput").ap(),
        "wd": nc.dram_tensor("wd", [EL, 4, P, IK, 512], BF16, kind="ExternalInput").ap(),
        "outS": nc.dram_tensor("outS", [P, HK, 512], BF16, kind="ExternalOutput").ap(),
    }
    for j in range(EL):
        d[f"xg{j}"] = nc.dram_tensor(f"xg{j}", [P, HK, caps[j]], BF16,
                                     kind="ExternalInput").ap()
        d[f"y{j}"] = nc.dram_tensor(f"y{j}", [P, HK, caps[j]], BF16,
                                    kind="ExternalOutput").ap()
    with tile.TileContext(nc) as tc:
        with (
            tc.tile_pool(name="sb", bufs=1) as sb,
            tc.tile_pool(name="work", bufs=2) as work,
            tc.tile_pool(name="wstream", bufs=3) as wstream,
            tc.tile_pool(name="ps", bufs=8, space="PSUM") as ps,
        ):
            pools = (sb, work, wstream, ps)
            if repeat == 1:
                _build_body(tc, d, pools, caps)
            else:
                with tc.For_i(0, repeat, 1):
                    _build_body(tc, d, pools, caps)
    nc.compile()
    return nc


_NC_CACHE = {}


def kernel(hidden_states, gate_w, gate_bias, w_gate_up, w_down,
           shared_w_gate_up, shared_w_down):
    caps, in_maps, meta = prepare(hidden_states, gate_w, gate_bias,
                                  w_gate_up, w_down,
                                  shared_w_gate_up, shared_w_down)
    if caps not in _NC_CACHE:
        _NC_CACHE[caps] = build_nc(caps, repeat=1)
    nc = _NC_CACHE[caps]
    res = run_bass_kernel_spmd(nc, in_maps, list(range(NCORES)))
    return combine(res.results, caps, meta)
